# revision 27
# baseline (speedup 1.0000x reference)
"""GNN message-passing (SpMM + mean-normalize + bias) Trainium2 kernel.

out[r] = (sum_{e: rows[e]==r} vals[e] * x[cols[e]]) / deg[r] + bias,
deg[r] = sum vals[e], rows with deg==0 -> bias.

Strategy (8 NeuronCores, SPMD):
  - Pad N=40000 rows to 40960 = 320 bins x 128 rows. Core c owns bins
    [40c, 40c+40) => output rows [5120c, 5120(c+1)).  Edges are bucketed by
    destination bin on the host (this is the sharding step), so no
    cross-core collectives are needed.
  - Per bin, edges are split into a low group (col < 32768) and a high
    group (col >= 32768) (dma_gather carries int16 indices), sorted by
    column for HBM locality, and padded to a multiple of 128 with null
    edges (val=0).
  - x is converted to bf16 on the host; dma_gather fetches 256B rows, slot
    i <- (partition i%128, chunk i//128).  The one-hot selection matrices
    S[t, r] = (ri[t]==r)*val[t] (bf16) are precomputed on the host from the
    edge structure and streamed in with plain (HWDGE) DMAs, so neither the
    vector engine nor extra Q7 work is needed.  Per chunk the tensor engine
    computes psum[r,f] += S^T @ xg (bf16 matmul, fp32 PSUM accumulate); the
    deg*bias outer product seeds the accumulator.  Epilogue scales by
    1/deg (fp32) on ACT and DMAs the 128-row block out.
"""
import sys

sys.path.insert(0, "/opt/trn_rl_repo")

import numpy as np
from ml_dtypes import bfloat16

N_NODES = 40000
N_EDGES = 640000
D = 128
P = 128
N_CORES = 8
BINS_PER_CORE = 40
N_BINS = N_CORES * BINS_PER_CORE          # 320 (rows padded to 40960)
SPLIT = 32768                             # int16-safe index split

_plan_cache: dict = {}


def _build_program(NLO, NHI, NXL, NXH):
    """Build+compile the SPMD Bass program for the given per-bin-position
    chunk schedule (shared by all cores)."""
    import concourse.bacc as bacc
    import concourse.bass as bass
    import concourse.tile as tile
    from concourse import mybir

    NCH = [NLO[p] + NHI[p] for p in range(BINS_PER_CORE)]
    F = sum(NCH)
    # process bins largest-first (shortest tail); idx for the first few
    # processed bins is a separate small tensor so gathers start immediately
    PORDER = _porder(NCH)
    OFF = {}
    _cum = 0
    for p in PORDER:
        OFF[p] = _cum
        _cum += NCH[p]
    FA = sum(NCH[p] for p in PORDER[:IDXA_BINS])

    NQ = 4
    nc = bacc.Bacc(num_swdge_queues=NQ)
    x_d = nc.dram_tensor("x", [N_NODES, D], mybir.dt.bfloat16,
                         kind="ExternalInput")
    idxa_d = nc.dram_tensor("idxa", [P, FA * 8], mybir.dt.int16,
                            kind="ExternalInput")
    idxb_d = nc.dram_tensor("idxb", [P, (F - FA) * 8], mybir.dt.int16,
                            kind="ExternalInput")
    sall_d = nc.dram_tensor("sall", [P, F * P], mybir.dt.bfloat16,
                            kind="ExternalInput")
    biasrow_d = nc.dram_tensor("biasrow", [1, D], mybir.dt.bfloat16,
                               kind="ExternalInput")
    out_d = nc.dram_tensor("out", [BINS_PER_CORE * P, D], mybir.dt.float32,
                           kind="ExternalOutput")

    with tile.TileContext(nc) as tc:
        with tc.tile_pool(name="persist", bufs=1) as persist, \
             tc.tile_pool(name="xgp", bufs=5) as xgp, \
             tc.tile_pool(name="spool", bufs=4) as spool, \
             tc.tile_pool(name="outp", bufs=3) as outp, \
             tc.tile_pool(name="ps", bufs=4, space="PSUM") as ps:
            idxa_t = persist.tile([P, FA * 8], mybir.dt.int16)
            idxb_t = persist.tile([P, (F - FA) * 8], mybir.dt.int16)
            biasrow_t = persist.tile([1, D], mybir.dt.bfloat16)
            ones_t = persist.tile([1, P], mybir.dt.bfloat16)
            nc.sync.dma_start(out=idxa_t[:], in_=idxa_d[:, :])
            nc.scalar.dma_start(out=biasrow_t[:], in_=biasrow_d[:, :])
            nc.vector.memset(ones_t[:], 1.0)
            nc.sync.dma_start(out=idxb_t[:], in_=idxb_d[:, :])

            maxch = max(NCH)
            for _w in range(5):
                wt = xgp.tile([P, maxch * D], mybir.dt.bfloat16, tag="xg")
                nc.vector.memset(wt[:], 0.0)
            # dma_gather is limited to 1024 indices (8 chunks) per call:
            # larger calls (tried 1536/1920/2048) hang or fail at runtime
            GMAX = 8
            _gq = [0]
            for k, b in enumerate(PORDER):
                offb = OFF[b]
                nch, nlo, nhi = NCH[b], NLO[b], NHI[b]
                if k < IDXA_BINS:
                    idx_t, ioff = idxa_t, offb
                else:
                    idx_t, ioff = idxb_t, offb - FA
                xg = xgp.tile([P, nch * D], mybir.dt.bfloat16, tag="xg")
                subs = []  # (chunk off, n chunks, is_high, exact idx count)
                for s in range(0, nlo, GMAX):
                    n = min(GMAX, nlo - s)
                    nidx = max(16, min(n * 128, NXL[b] - s * 128))
                    subs.append((s, n, False, nidx))
                for s in range(0, nhi, GMAX):
                    n = min(GMAX, nhi - s)
                    nidx = max(16, min(n * 128, NXH[b] - s * 128))
                    subs.append((nlo + s, n, True, nidx))
                for s, n, hi, nidx in subs:
                    nc.gpsimd.dma_gather(
                        out_ap=xg[:, s * D : (s + n) * D].rearrange(
                            "p (k w) -> p k w", k=n),
                        in_ap=(x_d[SPLIT:N_NODES, :] if hi else x_d[0:SPLIT, :]),
                        idxs_ap=idx_t[:, (ioff + s) * 8 : (ioff + s + n) * 8],
                        num_idxs=nidx,
                        num_idxs_reg=nidx,
                        elem_size=D,
                        queue_num=_gq[0] % NQ,
                    )
                    _gq[0] += 1
                # stream this bin's S blocks (alternate HWDGE issuers)
                s_t = spool.tile([P, nch * P], mybir.dt.bfloat16, tag="S")
                seng = nc.scalar if (k % 2) else nc.sync
                seng.dma_start(out=s_t[:],
                               in_=sall_d[:, offb * P : (offb + nch) * P])
                psum = ps.tile([P, D], mybir.dt.float32, tag="psum")
                # seed psum[r, f] = bias[f]; 1/deg is folded into S host-side
                nc.tensor.matmul(out=psum[:], lhsT=ones_t[:, :],
                                 rhs=biasrow_t[:, :],
                                 start=True, stop=False)
                for c in range(nch):
                    nc.tensor.matmul(out=psum[:],
                                     lhsT=s_t[:, c * P : (c + 1) * P],
                                     rhs=xg[:, c * D : (c + 1) * D],
                                     start=False, stop=(c == nch - 1))
                # epilogue: psum already holds agg/deg + bias; copy out on ACT
                o_t = outp.tile([P, D], mybir.dt.float32, tag="o")
                nc.scalar.activation(
                    out=o_t[:], in_=psum[:],
                    func=mybir.ActivationFunctionType.Copy)
                nc.scalar.dma_start(out=out_d[b * P : (b + 1) * P, :],
                                    in_=o_t[:])

    nc.compile()
    return nc


def _cdiv(a, b):
    return -(-a // b)


IDXA_BINS = 6


def _porder(NCH):
    """Bin processing order: largest first, smallest last (short tail)."""
    return sorted(range(BINS_PER_CORE), key=lambda p: (-NCH[p], p))


def _preprocess(x, edge_rows, edge_cols, adj_vals, bias):
    """Bucket edges by destination bin, split low/high cols, sort by col,
    pad, and build per-core device input arrays."""
    deg = np.bincount(edge_rows, weights=adj_vals.astype(np.float64),
                      minlength=N_BINS * P).astype(np.float32)
    rdeg = np.ones(N_BINS * P, np.float32)
    nz = deg != 0
    rdeg[nz] = (1.0 / deg[nz]).astype(np.float32)

    bin_id = (edge_rows // P).astype(np.int64)
    is_high = (edge_cols >= SPLIT).astype(np.int64)
    order = np.lexsort((edge_cols, is_high, bin_id))
    b_s = bin_id[order]
    h_s = is_high[order]
    col_s = edge_cols[order].astype(np.int32)
    # fold mean-normalization into the edge weights
    val_s = (adj_vals[order] * rdeg[edge_rows[order]]).astype(np.float32)
    ri_s = (edge_rows[order] - b_s * P).astype(np.int64)

    n_tot = np.bincount(b_s, minlength=N_BINS)
    n_hi = np.bincount(b_s, weights=h_s, minlength=N_BINS).astype(np.int64)
    n_lo = n_tot - n_hi
    starts = np.concatenate([[0], np.cumsum(n_tot)])[:N_BINS]

    # per-(bin, half) unique columns: one gather slot per distinct column,
    # S maps the slot to every row using it
    UQ = {}
    n_lo_u = np.zeros(N_BINS, np.int64)
    n_hi_u = np.zeros(N_BINS, np.int64)
    for g in range(N_BINS):
        s = int(starts[g])
        nl, nh = int(n_lo[g]), int(n_hi[g])
        u_lo, inv_lo = np.unique(col_s[s : s + nl], return_inverse=True)
        u_hi, inv_hi = np.unique(col_s[s + nl : s + nl + nh],
                                 return_inverse=True)
        UQ[g] = (u_lo, inv_lo, u_hi, inv_hi)
        n_lo_u[g], n_hi_u[g] = len(u_lo), len(u_hi)

    # per-position chunk counts, shared across cores (SPMD)
    NLO = [max(1, int(max(_cdiv(int(n_lo_u[40 * c + p]), P)
                          for c in range(N_CORES))))
           for p in range(BINS_PER_CORE)]
    NHI = [max(1, int(max(_cdiv(int(n_hi_u[40 * c + p]), P)
                          for c in range(N_CORES))))
           for p in range(BINS_PER_CORE)]
    NCH = [NLO[p] + NHI[p] for p in range(BINS_PER_CORE)]
    F = sum(NCH)
    NXL = [max(16, 16 * int(_cdiv(int(max(n_lo_u[40 * c + p] for c in range(N_CORES))), 16)))
           for p in range(BINS_PER_CORE)]
    NXH = [max(16, 16 * int(_cdiv(int(max(n_hi_u[40 * c + p] for c in range(N_CORES))), 16)))
           for p in range(BINS_PER_CORE)]

    x_bf = np.ascontiguousarray(x, dtype=np.float32).astype(bfloat16)

    NCH_l = [NLO[p] + NHI[p] for p in range(BINS_PER_CORE)]
    porder = _porder(NCH_l)
    FA = sum(NCH_l[p] for p in porder[:IDXA_BINS])

    in_maps = []
    for c in range(N_CORES):
        idx_parts = []
        # S_all[t, chunk*P + r] = sum of vals of edges whose column sits in
        # gather slot (t, chunk) and whose in-bin row is r
        pos_parts, sval_parts = [], []
        off = 0
        for p in porder:
            g = 40 * c + p
            s = int(starts[g])
            nl, nh = int(n_lo[g]), int(n_hi[g])
            u_lo, inv_lo, u_hi, inv_hi = UQ[g]
            nlu, nhu = len(u_lo), len(u_hi)
            lo_pad, hi_pad = NLO[p] * P, NHI[p] * P
            cols_lo = np.zeros(lo_pad, np.int32)
            cols_lo[:nlu] = u_lo
            cols_hi = np.full(hi_pad, SPLIT, np.int32)
            cols_hi[:nhu] = u_hi
            # wrapped int16 idx layout: idx i at [i%16, i//16], replicated 8x
            wlo = cols_lo.reshape(-1, 16).T.astype(np.int16)
            whi = (cols_hi - SPLIT).reshape(-1, 16).T.astype(np.int16)
            idx_parts.append(np.tile(wlo, (8, 1)))
            idx_parts.append(np.tile(whi, (8, 1)))
            # edge -> slot (its unique col) -> t = slot%128, chunk = slot//128
            nch = NCH[p]
            for blk, inv, base, cnt in ((0, inv_lo, s, nl),
                                        (lo_pad, inv_hi, s + nl, nh)):
                if cnt == 0:
                    continue
                slot = blk + inv
                t = slot % P
                ch = off + slot // P
                pos_parts.append(t * (F * P) + ch * P
                                 + ri_s[base : base + cnt])
                sval_parts.append(val_s[base : base + cnt])
            off += nch
        idx_np = np.concatenate(idx_parts, axis=1)
        s_flat = np.bincount(np.concatenate(pos_parts),
                             weights=np.concatenate(sval_parts),
                             minlength=P * F * P)
        sall_np = s_flat.reshape(P, F * P).astype(bfloat16)
        in_maps.append({
            "x": x_bf,
            "idxa": np.ascontiguousarray(idx_np[:, : FA * 8]),
            "idxb": np.ascontiguousarray(idx_np[:, FA * 8 :]),
            "sall": sall_np,
            "biasrow": np.asarray(bias, np.float32).reshape(1, -1).astype(
                bfloat16),
        })
    return tuple(NLO), tuple(NHI), tuple(NXL), tuple(NXH), in_maps


def _run(x, edge_rows, edge_cols, adj_vals, bias, trace=False, trace_cores=None):
    from concourse.bass_utils import run_bass_kernel_spmd

    NLO, NHI, NXL, NXH, in_maps = _preprocess(x, edge_rows, edge_cols,
                                              adj_vals, bias)
    key = (NLO, NHI, NXL, NXH)
    if key not in _plan_cache:
        _plan_cache[key] = _build_program(list(NLO), list(NHI), list(NXL),
                                          list(NXH))
    nc = _plan_cache[key]
    kw = {}
    if trace:
        kw["trace"] = True
        if trace_cores is not None:
            kw["trace_cores"] = trace_cores
    res = run_bass_kernel_spmd(nc, in_maps, core_ids=list(range(N_CORES)), **kw)
    out = np.concatenate([res.results[c]["out"] for c in range(N_CORES)], axis=0)
    return out[:N_NODES].astype(np.float32), res


def kernel(x, edge_rows, edge_cols, adj_vals, bias):
    out, _ = _run(np.asarray(x), np.asarray(edge_rows), np.asarray(edge_cols),
                  np.asarray(adj_vals), np.asarray(bias))
    return out


# revision 28
# speedup vs baseline: 1.1631x; 1.1631x over previous
"""GNN message-passing (SpMM + mean-normalize + bias) Trainium2 kernel.

out[r] = (sum_{e: rows[e]==r} vals[e] * x[cols[e]]) / deg[r] + bias,
deg[r] = sum vals[e], rows with deg==0 -> bias.

Strategy (8 NeuronCores, SPMD):
  - Pad N=40000 rows to 40960 = 320 bins x 128 rows. Core c owns bins
    [40c, 40c+40) => output rows [5120c, 5120(c+1)).  Edges are bucketed by
    destination bin on the host (this is the sharding step), so no
    cross-core collectives are needed.
  - Per bin, the distinct columns (one gather slot per distinct column,
    sorted for HBM locality) are split into ~2 balanced groups of <=1024
    slots.  Group bases exploit the int16 gather index: a low group reads
    x[idx] (all its cols < 32768), a high group reads x[7232 + idx] (all
    its cols >= 7232) -- one dma_gather call per group.
  - x is converted to bf16 on the host; dma_gather fetches 256B rows, slot
    i <- (partition i%128, chunk i//128).  The selection matrices
    S[t, r] = sum of val/deg over edges (col in slot t -> row r) (bf16) are
    precomputed on the host from the edge structure and streamed in with
    plain HWDGE DMAs, so neither the vector engine nor extra Q7 work is
    needed.  Per chunk the tensor engine computes psum[r,f] += S^T @ xg
    (bf16 matmul, fp32 PSUM accumulate); a ones x bias outer product seeds
    the accumulator.  Epilogue copies PSUM out on ACT (mean-normalization
    is folded into S on the host).
"""
import sys

sys.path.insert(0, "/opt/trn_rl_repo")

import numpy as np
from ml_dtypes import bfloat16

N_NODES = 40000
N_EDGES = 640000
D = 128
P = 128
N_CORES = 8
BINS_PER_CORE = 40
N_BINS = N_CORES * BINS_PER_CORE          # 320 (rows padded to 40960)
SPLIT = 32768                             # int16 index reach
HIBASE = N_NODES - SPLIT                  # 7232: base for high groups
GSLOTS = 1024                             # max slots per gather call

_plan_cache: dict = {}


def _build_program(GROUPS):
    """Build+compile the SPMD Bass program.  GROUPS[p] is a tuple of
    (nchunks, is_high, nidx) per gather group of bin-position p, shared by
    all cores."""
    import concourse.bacc as bacc
    import concourse.bass as bass
    import concourse.tile as tile
    from concourse import mybir

    NCH = [sum(g[0] for g in GROUPS[p]) for p in range(BINS_PER_CORE)]
    F = sum(NCH)
    F16 = F * 8

    NQ = 4
    nc = bacc.Bacc(num_swdge_queues=NQ)
    x_d = nc.dram_tensor("x", [N_NODES, D], mybir.dt.bfloat16,
                         kind="ExternalInput")
    idx_d = nc.dram_tensor("idx", [P, F16], mybir.dt.int16, kind="ExternalInput")
    sall_d = nc.dram_tensor("sall", [P, F * P], mybir.dt.bfloat16,
                            kind="ExternalInput")
    biasrow_d = nc.dram_tensor("biasrow", [1, D], mybir.dt.bfloat16,
                               kind="ExternalInput")
    out_d = nc.dram_tensor("out", [BINS_PER_CORE * P, D], mybir.dt.float32,
                           kind="ExternalOutput")

    with tile.TileContext(nc) as tc:
        with tc.tile_pool(name="persist", bufs=1) as persist, \
             tc.tile_pool(name="xgp", bufs=5) as xgp, \
             tc.tile_pool(name="spool", bufs=4) as spool, \
             tc.tile_pool(name="outp", bufs=3) as outp, \
             tc.tile_pool(name="ps", bufs=4, space="PSUM") as ps, \
             tc.tile_pool(name="psd", bufs=2, space="PSUM") as psd:
            idx_t = persist.tile([P, F16], mybir.dt.int16)
            biasrow_t = persist.tile([1, D], mybir.dt.bfloat16)
            ones_t = persist.tile([1, P], mybir.dt.bfloat16)
            nc.sync.dma_start(out=idx_t[:], in_=idx_d[:, :])
            nc.sync.dma_start(out=biasrow_t[:], in_=biasrow_d[:, :])
            nc.vector.memset(ones_t[:], 1.0)

            maxch = max(NCH)
            for _w in range(5):
                wt = xgp.tile([P, maxch * D], mybir.dt.bfloat16, tag="xg")
                nc.vector.memset(wt[:], 0.0)
            _gq = [0]
            for b in range(BINS_PER_CORE):
                offb = sum(NCH[:b])
                nch = NCH[b]
                xg = xgp.tile([P, nch * D], mybir.dt.bfloat16, tag="xg")
                subs = []  # (chunk off, n chunks, is_high, exact idx count)
                s = 0
                for n, hi, nidx in GROUPS[b]:
                    subs.append((s, n, hi, nidx))
                    s += n
                for s, n, hi, nidx in subs:
                    nc.gpsimd.dma_gather(
                        out_ap=xg[:, s * D : (s + n) * D].rearrange(
                            "p (k w) -> p k w", k=n),
                        in_ap=(x_d[HIBASE:N_NODES, :] if hi
                               else x_d[0:SPLIT, :]),
                        idxs_ap=idx_t[:, (offb + s) * 8 : (offb + s + n) * 8],
                        num_idxs=nidx,
                        num_idxs_reg=nidx,
                        elem_size=D,
                        queue_num=_gq[0] % NQ,
                    )
                    _gq[0] += 1
                # stream this bin's S blocks (alternate HWDGE issuers)
                s_t = spool.tile([P, nch * P], mybir.dt.bfloat16, tag="S")
                seng = nc.scalar if (b % 2) else nc.sync
                seng.dma_start(out=s_t[:],
                               in_=sall_d[:, offb * P : (offb + nch) * P])
                psum = ps.tile([P, D], mybir.dt.float32, tag="psum")
                # seed psum[r, f] = bias[f]; 1/deg is folded into S host-side
                nc.tensor.matmul(out=psum[:], lhsT=ones_t[:, :],
                                 rhs=biasrow_t[:, :],
                                 start=True, stop=False)
                # tiny PE reads of xg: absorb the gather-DMA semaphore waits
                # so real matmuls carry only the S-load wait
                dummy = psd.tile([1, 1], mybir.dt.float32, tag="dummy")
                for s, n, hi, nidx in subs:
                    nc.tensor.matmul(out=dummy[:], lhsT=xg[:1, s * D : s * D + 1],
                                     rhs=xg[:1, s * D : s * D + 1],
                                     start=True, stop=True)
                for c in range(nch):
                    nc.tensor.matmul(out=psum[:],
                                     lhsT=s_t[:, c * P : (c + 1) * P],
                                     rhs=xg[:, c * D : (c + 1) * D],
                                     start=False, stop=(c == nch - 1))
                # epilogue: psum already holds agg/deg + bias; copy out on ACT
                o_t = outp.tile([P, D], mybir.dt.float32, tag="o")
                nc.scalar.activation(
                    out=o_t[:], in_=psum[:],
                    func=mybir.ActivationFunctionType.Copy)
                nc.sync.dma_start(out=out_d[b * P : (b + 1) * P, :], in_=o_t[:])

    nc.compile()
    return nc


def _cdiv(a, b):
    return -(-a // b)


def _preprocess(x, edge_rows, edge_cols, adj_vals, bias):
    """Bucket edges by destination bin, dedup columns, split into balanced
    gather groups, and build per-core device input arrays."""
    deg = np.bincount(edge_rows, weights=adj_vals.astype(np.float64),
                      minlength=N_BINS * P).astype(np.float32)
    rdeg = np.ones(N_BINS * P, np.float32)
    nz = deg != 0
    rdeg[nz] = (1.0 / deg[nz]).astype(np.float32)

    bin_id = (edge_rows // P).astype(np.int64)
    order = np.lexsort((edge_cols, bin_id))
    b_s = bin_id[order]
    col_s = edge_cols[order].astype(np.int32)
    # fold mean-normalization into the edge weights
    val_s = (adj_vals[order] * rdeg[edge_rows[order]]).astype(np.float32)
    ri_s = (edge_rows[order] - b_s * P).astype(np.int64)

    n_tot = np.bincount(b_s, minlength=N_BINS)
    starts = np.concatenate([[0], np.cumsum(n_tot)])[:N_BINS]

    # per-(core,bin) distinct sorted columns; one gather slot per distinct col
    UQ = {}
    n_u = np.zeros(N_BINS, np.int64)
    for g in range(N_BINS):
        s = int(starts[g])
        cnt = int(n_tot[g])
        u, inv = np.unique(col_s[s : s + cnt], return_inverse=True)
        UQ[g] = (u, inv)
        n_u[g] = len(u)

    # shared schedule per bin position: ngroups, per-group chunk counts,
    # group bases, and exact gather counts (max over cores, 16-aligned)
    GROUPS = []
    for p in range(BINS_PER_CORE):
        cores_u = [int(n_u[40 * c + p]) for c in range(N_CORES)]
        u_max = max(max(cores_u), 1)
        ng = max(1, _cdiv(u_max, GSLOTS))
        while True:
            ok = True
            glist = []
            for j in range(ng):
                # per-core balanced split [j*u/ng, (j+1)*u/ng)
                lens, lo_cols, hi_cols = [], [], []
                for c in range(N_CORES):
                    u, _ = UQ[40 * c + p]
                    a = (len(u) * j) // ng
                    bnd = (len(u) * (j + 1)) // ng
                    lens.append(bnd - a)
                    if bnd > a:
                        lo_cols.append(int(u[a]))
                        hi_cols.append(int(u[bnd - 1]))
                maxlen = max(max(lens), 1)
                if maxlen > GSLOTS:
                    ok = False
                    break
                cmax = max(hi_cols) if hi_cols else 0
                cmin = min(lo_cols) if lo_cols else 0
                if cmax < SPLIT:
                    hi = False
                elif cmin >= HIBASE:
                    hi = True
                else:
                    ok = False
                    break
                nidx = max(16, 16 * _cdiv(maxlen, 16))
                glist.append((_cdiv(maxlen, P), hi, nidx))
            if ok:
                GROUPS.append(tuple(glist))
                break
            ng += 1
            assert ng <= 8, f"bin position {p}: cannot partition columns"
    NCH = [sum(g[0] for g in GROUPS[p]) for p in range(BINS_PER_CORE)]
    F = sum(NCH)

    x_bf = np.ascontiguousarray(x, dtype=np.float32).astype(bfloat16)

    in_maps = []
    for c in range(N_CORES):
        idx_parts = []
        # S_all[t, chunk*P + r] = sum of vals of edges whose column sits in
        # gather slot (t, chunk) and whose in-bin row is r
        pos_parts, sval_parts = [], []
        off = 0
        for p in range(BINS_PER_CORE):
            g = 40 * c + p
            s = int(starts[g])
            cnt = int(n_tot[g])
            u, inv = UQ[g]
            ng = len(GROUPS[p])
            # slot index of each distinct col: group-local position plus
            # padded offset of its group
            slot_of_u = np.empty(max(len(u), 1), np.int64)
            goff = 0
            for j, (nchj, hi, nidx) in enumerate(GROUPS[p]):
                a = (len(u) * j) // ng
                bnd = (len(u) * (j + 1)) // ng
                base = HIBASE if hi else 0
                padded = np.zeros(nchj * P, np.int32)
                if bnd > a:
                    gi = u[a:bnd] - base
                    assert gi.min() >= 0 and gi.max() < SPLIT, (p, j)
                    padded[: bnd - a] = gi
                    slot_of_u[a:bnd] = goff + np.arange(bnd - a)
                # wrapped int16 idx layout: idx i at [i%16, i//16], x8
                w = padded.reshape(-1, 16).T.astype(np.int16)
                idx_parts.append(np.tile(w, (8, 1)))
                goff += nchj * P
            nch = NCH[p]
            if cnt:
                slot = slot_of_u[inv]
                t = slot % P
                ch = off + slot // P
                pos_parts.append(t * (F * P) + ch * P + ri_s[s : s + cnt])
                sval_parts.append(val_s[s : s + cnt])
            off += nch
        idx_np = np.concatenate(idx_parts, axis=1)
        s_flat = np.bincount(np.concatenate(pos_parts),
                             weights=np.concatenate(sval_parts),
                             minlength=P * F * P)
        sall_np = s_flat.reshape(P, F * P).astype(bfloat16)
        in_maps.append({
            "x": x_bf,
            "idx": idx_np,
            "sall": sall_np,
            "biasrow": np.asarray(bias, np.float32).reshape(1, -1).astype(
                bfloat16),
        })
    return tuple(GROUPS), in_maps


def _run(x, edge_rows, edge_cols, adj_vals, bias, trace=False, trace_cores=None):
    from concourse.bass_utils import run_bass_kernel_spmd

    GROUPS, in_maps = _preprocess(x, edge_rows, edge_cols, adj_vals, bias)
    if GROUPS not in _plan_cache:
        _plan_cache[GROUPS] = _build_program(list(GROUPS))
    nc = _plan_cache[GROUPS]
    kw = {}
    if trace:
        kw["trace"] = True
        if trace_cores is not None:
            kw["trace_cores"] = trace_cores
    res = run_bass_kernel_spmd(nc, in_maps, core_ids=list(range(N_CORES)), **kw)
    out = np.concatenate([res.results[c]["out"] for c in range(N_CORES)], axis=0)
    return out[:N_NODES].astype(np.float32), res


def kernel(x, edge_rows, edge_cols, adj_vals, bias):
    out, _ = _run(np.asarray(x), np.asarray(edge_rows), np.asarray(edge_cols),
                  np.asarray(adj_vals), np.asarray(bias))
    return out


# revision 33
# speedup vs baseline: 1.1746x; 1.0099x over previous
"""GNN message-passing (SpMM + mean-normalize + bias) Trainium2 kernel.

out[r] = (sum_{e: rows[e]==r} vals[e] * x[cols[e]]) / deg[r] + bias,
deg[r] = sum vals[e], rows with deg==0 -> bias.

Strategy (8 NeuronCores, SPMD):
  - Pad N=40000 rows to 40960 = 320 bins x 128 rows. Core c owns bins
    [40c, 40c+40) => output rows [5120c, 5120(c+1)).  Edges are bucketed by
    destination bin on the host (this is the sharding step), so no
    cross-core collectives are needed.
  - Per bin, the distinct columns (one gather slot per distinct column,
    sorted for HBM locality) are split into ~2 balanced groups of <=1024
    slots.  Group bases exploit the int16 gather index: a low group reads
    x[idx] (all its cols < 32768), a high group reads x[7232 + idx] (all
    its cols >= 7232) -- one dma_gather call per group.
  - x is converted to bf16 on the host; dma_gather fetches 256B rows, slot
    i <- (partition i%128, chunk i//128).  The selection matrices
    S[t, r] = sum of val/deg over edges (col in slot t -> row r) (bf16) are
    precomputed on the host from the edge structure and streamed in with
    plain HWDGE DMAs, so neither the vector engine nor extra Q7 work is
    needed.  Per chunk the tensor engine computes psum[r,f] += S^T @ xg
    (bf16 matmul, fp32 PSUM accumulate); a ones x bias outer product seeds
    the accumulator.  Epilogue copies PSUM out on ACT (mean-normalization
    is folded into S on the host).
"""
import sys

sys.path.insert(0, "/opt/trn_rl_repo")

import numpy as np
from ml_dtypes import bfloat16

N_NODES = 40000
N_EDGES = 640000
D = 128
P = 128
N_CORES = 8
BINS_PER_CORE = 40
N_BINS = N_CORES * BINS_PER_CORE          # 320 (rows padded to 40960)
SPLIT = 32768                             # int16 index reach
HIBASE = N_NODES - SPLIT                  # 7232: base for high groups
GSLOTS = 1024                             # max slots per gather call
IDXA_BINS = 6                             # bins covered by the small idx load

_plan_cache: dict = {}


def _build_program(GROUPS):
    """Build+compile the SPMD Bass program.  GROUPS[p] is a tuple of
    (nchunks, is_high, nidx) per gather group of bin-position p, shared by
    all cores."""
    import concourse.bacc as bacc
    import concourse.bass as bass
    import concourse.tile as tile
    from concourse import mybir

    NCH = [sum(g[0] for g in GROUPS[p]) for p in range(BINS_PER_CORE)]
    F = sum(NCH)
    F16 = F * 8

    NQ = 4
    nc = bacc.Bacc(num_swdge_queues=NQ)
    # idx for the first IDXA_BINS bins is a separate small tensor so the
    # first gathers are not gated on the full idx-table load
    FA = sum(NCH[:IDXA_BINS])
    x_d = nc.dram_tensor("x", [N_NODES, D], mybir.dt.bfloat16,
                         kind="ExternalInput")
    idxa_d = nc.dram_tensor("idxa", [P, FA * 8], mybir.dt.int16,
                            kind="ExternalInput")
    idxb_d = nc.dram_tensor("idxb", [P, (F - FA) * 8], mybir.dt.int16,
                            kind="ExternalInput")
    sall_d = nc.dram_tensor("sall", [P, F * P], mybir.dt.bfloat16,
                            kind="ExternalInput")
    biasrow_d = nc.dram_tensor("biasrow", [1, D], mybir.dt.bfloat16,
                               kind="ExternalInput")
    out_d = nc.dram_tensor("out", [BINS_PER_CORE * P, D], mybir.dt.float32,
                           kind="ExternalOutput")

    with tile.TileContext(nc) as tc:
        with tc.tile_pool(name="persist", bufs=1) as persist, \
             tc.tile_pool(name="xgp", bufs=5) as xgp, \
             tc.tile_pool(name="spool", bufs=4) as spool, \
             tc.tile_pool(name="outp", bufs=3) as outp, \
             tc.tile_pool(name="ps", bufs=4, space="PSUM") as ps, \
             tc.tile_pool(name="psd", bufs=2, space="PSUM") as psd:
            idxa_t = persist.tile([P, FA * 8], mybir.dt.int16)
            idxb_t = persist.tile([P, (F - FA) * 8], mybir.dt.int16)
            biasrow_t = persist.tile([1, D], mybir.dt.bfloat16)
            ones_t = persist.tile([1, P], mybir.dt.bfloat16)
            nc.sync.dma_start(out=idxa_t[:], in_=idxa_d[:, :])
            nc.sync.dma_start(out=biasrow_t[:], in_=biasrow_d[:, :])
            nc.sync.dma_start(out=idxb_t[:], in_=idxb_d[:, :])
            nc.vector.memset(ones_t[:], 1.0)

            maxch = max(NCH)
            for _w in range(5):
                wt = xgp.tile([P, maxch * D], mybir.dt.bfloat16, tag="xg")
                nc.vector.memset(wt[:], 0.0)
            _gq = [0]
            for b in range(BINS_PER_CORE):
                offb = sum(NCH[:b])
                nch = NCH[b]
                xg = xgp.tile([P, nch * D], mybir.dt.bfloat16, tag="xg")
                subs = []  # (chunk off, n chunks, is_high, exact idx count)
                s = 0
                for n, hi, nidx in GROUPS[b]:
                    subs.append((s, n, hi, nidx))
                    s += n
                if b < IDXA_BINS:
                    idx_t, ioff = idxa_t, offb
                else:
                    idx_t, ioff = idxb_t, offb - FA
                for s, n, hi, nidx in subs:
                    nc.gpsimd.dma_gather(
                        out_ap=xg[:, s * D : (s + n) * D].rearrange(
                            "p (k w) -> p k w", k=n),
                        in_ap=(x_d[HIBASE:N_NODES, :] if hi
                               else x_d[0:SPLIT, :]),
                        idxs_ap=idx_t[:, (ioff + s) * 8 : (ioff + s + n) * 8],
                        num_idxs=nidx,
                        num_idxs_reg=nidx,
                        elem_size=D,
                        queue_num=_gq[0] % NQ,
                    )
                    _gq[0] += 1
                # stream this bin's S blocks (alternate HWDGE issuers)
                s_t = spool.tile([P, nch * P], mybir.dt.bfloat16, tag="S")
                seng = nc.scalar if (b % 2) else nc.sync
                seng.dma_start(out=s_t[:],
                               in_=sall_d[:, offb * P : (offb + nch) * P])
                psum = ps.tile([P, D], mybir.dt.float32, tag="psum")
                # seed psum[r, f] = bias[f]; 1/deg is folded into S host-side
                nc.tensor.matmul(out=psum[:], lhsT=ones_t[:, :],
                                 rhs=biasrow_t[:, :],
                                 start=True, stop=False)
                # tiny PE reads of xg: absorb the gather-DMA semaphore waits
                # so real matmuls carry only the S-load wait
                dummy = psd.tile([1, 1], mybir.dt.float32, tag="dummy")
                for s, n, hi, nidx in subs:
                    nc.tensor.matmul(out=dummy[:], lhsT=xg[:1, s * D : s * D + 1],
                                     rhs=xg[:1, s * D : s * D + 1],
                                     start=True, stop=True)
                for c in range(nch):
                    nc.tensor.matmul(out=psum[:],
                                     lhsT=s_t[:, c * P : (c + 1) * P],
                                     rhs=xg[:, c * D : (c + 1) * D],
                                     start=False, stop=(c == nch - 1))
                # epilogue: psum already holds agg/deg + bias; copy out on ACT
                o_t = outp.tile([P, D], mybir.dt.float32, tag="o")
                nc.scalar.activation(
                    out=o_t[:], in_=psum[:],
                    func=mybir.ActivationFunctionType.Copy)
                nc.sync.dma_start(out=out_d[b * P : (b + 1) * P, :], in_=o_t[:])

    nc.compile()
    return nc


def _cdiv(a, b):
    return -(-a // b)


def _preprocess(x, edge_rows, edge_cols, adj_vals, bias):
    """Bucket edges by destination bin, dedup columns, split into balanced
    gather groups, and build per-core device input arrays."""
    deg = np.bincount(edge_rows, weights=adj_vals.astype(np.float64),
                      minlength=N_BINS * P).astype(np.float32)
    rdeg = np.ones(N_BINS * P, np.float32)
    nz = deg != 0
    rdeg[nz] = (1.0 / deg[nz]).astype(np.float32)

    bin_id = (edge_rows // P).astype(np.int64)
    order = np.lexsort((edge_cols, bin_id))
    b_s = bin_id[order]
    col_s = edge_cols[order].astype(np.int32)
    # fold mean-normalization into the edge weights
    val_s = (adj_vals[order] * rdeg[edge_rows[order]]).astype(np.float32)
    ri_s = (edge_rows[order] - b_s * P).astype(np.int64)

    n_tot = np.bincount(b_s, minlength=N_BINS)
    starts = np.concatenate([[0], np.cumsum(n_tot)])[:N_BINS]

    # per-(core,bin) distinct sorted columns; one gather slot per distinct col
    UQ = {}
    n_u = np.zeros(N_BINS, np.int64)
    for g in range(N_BINS):
        s = int(starts[g])
        cnt = int(n_tot[g])
        u, inv = np.unique(col_s[s : s + cnt], return_inverse=True)
        UQ[g] = (u, inv)
        n_u[g] = len(u)

    # shared schedule per bin position: ngroups, per-group chunk counts,
    # group bases, and exact gather counts (max over cores, 16-aligned)
    GROUPS = []
    for p in range(BINS_PER_CORE):
        cores_u = [int(n_u[40 * c + p]) for c in range(N_CORES)]
        u_max = max(max(cores_u), 1)
        ng = max(1, _cdiv(u_max, GSLOTS))
        while True:
            ok = True
            glist = []
            for j in range(ng):
                # per-core balanced split [j*u/ng, (j+1)*u/ng)
                lens, lo_cols, hi_cols = [], [], []
                for c in range(N_CORES):
                    u, _ = UQ[40 * c + p]
                    a = (len(u) * j) // ng
                    bnd = (len(u) * (j + 1)) // ng
                    lens.append(bnd - a)
                    if bnd > a:
                        lo_cols.append(int(u[a]))
                        hi_cols.append(int(u[bnd - 1]))
                maxlen = max(max(lens), 1)
                if maxlen > GSLOTS:
                    ok = False
                    break
                cmax = max(hi_cols) if hi_cols else 0
                cmin = min(lo_cols) if lo_cols else 0
                if cmax < SPLIT:
                    hi = False
                elif cmin >= HIBASE:
                    hi = True
                else:
                    ok = False
                    break
                nidx = max(16, 16 * _cdiv(maxlen, 16))
                glist.append((_cdiv(maxlen, P), hi, nidx))
            if ok:
                GROUPS.append(tuple(glist))
                break
            ng += 1
            assert ng <= 8, f"bin position {p}: cannot partition columns"
    NCH = [sum(g[0] for g in GROUPS[p]) for p in range(BINS_PER_CORE)]
    F = sum(NCH)

    x_bf = np.ascontiguousarray(x, dtype=np.float32).astype(bfloat16)

    in_maps = []
    for c in range(N_CORES):
        idx_parts = []
        # S_all[t, chunk*P + r] = sum of vals of edges whose column sits in
        # gather slot (t, chunk) and whose in-bin row is r
        pos_parts, sval_parts = [], []
        off = 0
        for p in range(BINS_PER_CORE):
            g = 40 * c + p
            s = int(starts[g])
            cnt = int(n_tot[g])
            u, inv = UQ[g]
            ng = len(GROUPS[p])
            # slot index of each distinct col: group-local position plus
            # padded offset of its group
            slot_of_u = np.empty(max(len(u), 1), np.int64)
            goff = 0
            for j, (nchj, hi, nidx) in enumerate(GROUPS[p]):
                a = (len(u) * j) // ng
                bnd = (len(u) * (j + 1)) // ng
                base = HIBASE if hi else 0
                padded = np.zeros(nchj * P, np.int32)
                if bnd > a:
                    gi = u[a:bnd] - base
                    assert gi.min() >= 0 and gi.max() < SPLIT, (p, j)
                    padded[: bnd - a] = gi
                    slot_of_u[a:bnd] = goff + np.arange(bnd - a)
                # wrapped int16 idx layout: idx i at [i%16, i//16], x8
                w = padded.reshape(-1, 16).T.astype(np.int16)
                idx_parts.append(np.tile(w, (8, 1)))
                goff += nchj * P
            nch = NCH[p]
            if cnt:
                slot = slot_of_u[inv]
                t = slot % P
                ch = off + slot // P
                pos_parts.append(t * (F * P) + ch * P + ri_s[s : s + cnt])
                sval_parts.append(val_s[s : s + cnt])
            off += nch
        idx_np = np.concatenate(idx_parts, axis=1)
        s_flat = np.bincount(np.concatenate(pos_parts),
                             weights=np.concatenate(sval_parts),
                             minlength=P * F * P)
        sall_np = s_flat.reshape(P, F * P).astype(bfloat16)
        FA = sum(NCH[:IDXA_BINS])
        in_maps.append({
            "x": x_bf,
            "idxa": np.ascontiguousarray(idx_np[:, : FA * 8]),
            "idxb": np.ascontiguousarray(idx_np[:, FA * 8 :]),
            "sall": sall_np,
            "biasrow": np.asarray(bias, np.float32).reshape(1, -1).astype(
                bfloat16),
        })
    return tuple(GROUPS), in_maps


def _run(x, edge_rows, edge_cols, adj_vals, bias, trace=False, trace_cores=None):
    from concourse.bass_utils import run_bass_kernel_spmd

    GROUPS, in_maps = _preprocess(x, edge_rows, edge_cols, adj_vals, bias)
    if GROUPS not in _plan_cache:
        _plan_cache[GROUPS] = _build_program(list(GROUPS))
    nc = _plan_cache[GROUPS]
    kw = {}
    if trace:
        kw["trace"] = True
        if trace_cores is not None:
            kw["trace_cores"] = trace_cores
    res = run_bass_kernel_spmd(nc, in_maps, core_ids=list(range(N_CORES)), **kw)
    out = np.concatenate([res.results[c]["out"] for c in range(N_CORES)], axis=0)
    return out[:N_NODES].astype(np.float32), res


def kernel(x, edge_rows, edge_cols, adj_vals, bias):
    out, _ = _run(np.asarray(x), np.asarray(edge_rows), np.asarray(edge_cols),
                  np.asarray(adj_vals), np.asarray(bias))
    return out


# revision 34
# speedup vs baseline: 1.1874x; 1.0109x over previous
"""GNN message-passing (SpMM + mean-normalize + bias) Trainium2 kernel.

out[r] = (sum_{e: rows[e]==r} vals[e] * x[cols[e]]) / deg[r] + bias,
deg[r] = sum vals[e], rows with deg==0 -> bias.

Strategy (8 NeuronCores, SPMD):
  - Pad N=40000 rows to 40960 = 320 bins x 128 rows. Core c owns bins
    [40c, 40c+40) => output rows [5120c, 5120(c+1)).  Edges are bucketed by
    destination bin on the host (this is the sharding step), so no
    cross-core collectives are needed.
  - Per bin, the distinct columns (one gather slot per distinct column,
    sorted for HBM locality) are split into ~2 balanced groups of <=1024
    slots.  Group bases exploit the int16 gather index: a low group reads
    x[idx] (all its cols < 32768), a high group reads x[7232 + idx] (all
    its cols >= 7232) -- one dma_gather call per group.
  - x is converted to bf16 on the host; dma_gather fetches 256B rows, slot
    i <- (partition i%128, chunk i//128).  The selection matrices
    S[t, r] = sum of val/deg over edges (col in slot t -> row r) (bf16) are
    precomputed on the host from the edge structure and streamed in with
    plain HWDGE DMAs, so neither the vector engine nor extra Q7 work is
    needed.  Per chunk the tensor engine computes psum[r,f] += S^T @ xg
    (bf16 matmul, fp32 PSUM accumulate); a ones x bias outer product seeds
    the accumulator.  Epilogue copies PSUM out on ACT (mean-normalization
    is folded into S on the host).
"""
import sys

sys.path.insert(0, "/opt/trn_rl_repo")

import numpy as np
from ml_dtypes import bfloat16

N_NODES = 40000
N_EDGES = 640000
D = 128
P = 128
N_CORES = 8
BINS_PER_CORE = 40
N_BINS = N_CORES * BINS_PER_CORE          # 320 (rows padded to 40960)
SPLIT = 32768                             # int16 index reach
HIBASE = N_NODES - SPLIT                  # 7232: base for high groups
GSLOTS = 1024                             # max slots per gather call
IDXA_BINS = 6                             # bins covered by the small idx load

_plan_cache: dict = {}


def _build_program(GROUPS):
    """Build+compile the SPMD Bass program.  GROUPS[p] is a tuple of
    (nchunks, is_high, nidx) per gather group of bin-position p, shared by
    all cores."""
    import concourse.bacc as bacc
    import concourse.bass as bass
    import concourse.tile as tile
    from concourse import mybir

    NCH = [sum(g[0] for g in GROUPS[p]) for p in range(BINS_PER_CORE)]
    F = sum(NCH)
    F16 = F * 8

    NQ = 4
    nc = bacc.Bacc(num_swdge_queues=NQ)
    # idx for the first IDXA_BINS bins is a separate small tensor so the
    # first gathers are not gated on the full idx-table load
    FA = sum(NCH[:IDXA_BINS])
    x_d = nc.dram_tensor("x", [N_NODES, D], mybir.dt.bfloat16,
                         kind="ExternalInput")
    idxa_d = nc.dram_tensor("idxa", [P, FA * 8], mybir.dt.int16,
                            kind="ExternalInput")
    idxb_d = nc.dram_tensor("idxb", [P, (F - FA) * 8], mybir.dt.int16,
                            kind="ExternalInput")
    sall_d = nc.dram_tensor("sall", [P, F * P], mybir.dt.bfloat16,
                            kind="ExternalInput")
    biasrow_d = nc.dram_tensor("biasrow", [1, D], mybir.dt.bfloat16,
                               kind="ExternalInput")
    out_d = nc.dram_tensor("out", [BINS_PER_CORE * P, D], mybir.dt.float32,
                           kind="ExternalOutput")

    with tile.TileContext(nc) as tc:
        with tc.tile_pool(name="persist", bufs=1) as persist, \
             tc.tile_pool(name="xgp", bufs=5) as xgp, \
             tc.tile_pool(name="spool", bufs=4) as spool, \
             tc.tile_pool(name="outp", bufs=3) as outp, \
             tc.tile_pool(name="ps", bufs=4, space="PSUM") as ps, \
             tc.tile_pool(name="psd", bufs=2, space="PSUM") as psd:
            # warm the gather buffers first (gathers leave padding slots
            # unwritten; stale SBUF bits could be inf/nan).  Split across
            # DVE and GpSimd so the first gather is not serialized behind
            # one engine's preamble.
            maxch = max(NCH)
            for _w in range(5):
                wt = xgp.tile([P, maxch * D], mybir.dt.bfloat16, tag="xg")
                eng = nc.gpsimd if _w % 2 else nc.vector
                eng.memset(wt[:], 0.0)

            idxa_t = persist.tile([P, FA * 8], mybir.dt.int16)
            idxb_t = persist.tile([P, (F - FA) * 8], mybir.dt.int16)
            biasrow_t = persist.tile([1, D], mybir.dt.bfloat16)
            ones_t = persist.tile([1, P], mybir.dt.bfloat16)
            nc.sync.dma_start(out=idxa_t[:], in_=idxa_d[:, :])
            nc.sync.dma_start(out=biasrow_t[:], in_=biasrow_d[:, :])
            nc.sync.dma_start(out=idxb_t[:], in_=idxb_d[:, :])
            nc.vector.memset(ones_t[:], 1.0)
            _gq = [0]
            for b in range(BINS_PER_CORE):
                offb = sum(NCH[:b])
                nch = NCH[b]
                xg = xgp.tile([P, nch * D], mybir.dt.bfloat16, tag="xg")
                subs = []  # (chunk off, n chunks, is_high, exact idx count)
                s = 0
                for n, hi, nidx in GROUPS[b]:
                    subs.append((s, n, hi, nidx))
                    s += n
                if b < IDXA_BINS:
                    idx_t, ioff = idxa_t, offb
                else:
                    idx_t, ioff = idxb_t, offb - FA
                for s, n, hi, nidx in subs:
                    nc.gpsimd.dma_gather(
                        out_ap=xg[:, s * D : (s + n) * D].rearrange(
                            "p (k w) -> p k w", k=n),
                        in_ap=(x_d[HIBASE:N_NODES, :] if hi
                               else x_d[0:SPLIT, :]),
                        idxs_ap=idx_t[:, (ioff + s) * 8 : (ioff + s + n) * 8],
                        num_idxs=nidx,
                        num_idxs_reg=nidx,
                        elem_size=D,
                        queue_num=_gq[0] % NQ,
                    )
                    _gq[0] += 1
                # stream this bin's S blocks (alternate HWDGE issuers)
                s_t = spool.tile([P, nch * P], mybir.dt.bfloat16, tag="S")
                seng = nc.scalar if (b % 2) else nc.sync
                seng.dma_start(out=s_t[:],
                               in_=sall_d[:, offb * P : (offb + nch) * P])
                psum = ps.tile([P, D], mybir.dt.float32, tag="psum")
                # seed psum[r, f] = bias[f]; 1/deg is folded into S host-side
                nc.tensor.matmul(out=psum[:], lhsT=ones_t[:, :],
                                 rhs=biasrow_t[:, :],
                                 start=True, stop=False)
                # tiny PE reads of xg: absorb the gather-DMA semaphore waits
                # so real matmuls carry only the S-load wait
                dummy = psd.tile([1, 1], mybir.dt.float32, tag="dummy")
                for s, n, hi, nidx in subs:
                    nc.tensor.matmul(out=dummy[:], lhsT=xg[:1, s * D : s * D + 1],
                                     rhs=xg[:1, s * D : s * D + 1],
                                     start=True, stop=True)
                for c in range(nch):
                    nc.tensor.matmul(out=psum[:],
                                     lhsT=s_t[:, c * P : (c + 1) * P],
                                     rhs=xg[:, c * D : (c + 1) * D],
                                     start=False, stop=(c == nch - 1))
                # epilogue: psum already holds agg/deg + bias; copy out on ACT
                o_t = outp.tile([P, D], mybir.dt.float32, tag="o")
                nc.scalar.activation(
                    out=o_t[:], in_=psum[:],
                    func=mybir.ActivationFunctionType.Copy)
                nc.sync.dma_start(out=out_d[b * P : (b + 1) * P, :], in_=o_t[:])

    nc.compile()
    return nc


def _cdiv(a, b):
    return -(-a // b)


def _preprocess(x, edge_rows, edge_cols, adj_vals, bias):
    """Bucket edges by destination bin, dedup columns, split into balanced
    gather groups, and build per-core device input arrays."""
    deg = np.bincount(edge_rows, weights=adj_vals.astype(np.float64),
                      minlength=N_BINS * P).astype(np.float32)
    rdeg = np.ones(N_BINS * P, np.float32)
    nz = deg != 0
    rdeg[nz] = (1.0 / deg[nz]).astype(np.float32)

    bin_id = (edge_rows // P).astype(np.int64)
    order = np.lexsort((edge_cols, bin_id))
    b_s = bin_id[order]
    col_s = edge_cols[order].astype(np.int32)
    # fold mean-normalization into the edge weights
    val_s = (adj_vals[order] * rdeg[edge_rows[order]]).astype(np.float32)
    ri_s = (edge_rows[order] - b_s * P).astype(np.int64)

    n_tot = np.bincount(b_s, minlength=N_BINS)
    starts = np.concatenate([[0], np.cumsum(n_tot)])[:N_BINS]

    # per-(core,bin) distinct sorted columns; one gather slot per distinct col
    UQ = {}
    n_u = np.zeros(N_BINS, np.int64)
    for g in range(N_BINS):
        s = int(starts[g])
        cnt = int(n_tot[g])
        u, inv = np.unique(col_s[s : s + cnt], return_inverse=True)
        UQ[g] = (u, inv)
        n_u[g] = len(u)

    # shared schedule per bin position: ngroups, per-group chunk counts,
    # group bases, and exact gather counts (max over cores, 16-aligned)
    GROUPS = []
    for p in range(BINS_PER_CORE):
        cores_u = [int(n_u[40 * c + p]) for c in range(N_CORES)]
        u_max = max(max(cores_u), 1)
        ng = max(1, _cdiv(u_max, GSLOTS))
        while True:
            ok = True
            glist = []
            for j in range(ng):
                # per-core balanced split [j*u/ng, (j+1)*u/ng)
                lens, lo_cols, hi_cols = [], [], []
                for c in range(N_CORES):
                    u, _ = UQ[40 * c + p]
                    a = (len(u) * j) // ng
                    bnd = (len(u) * (j + 1)) // ng
                    lens.append(bnd - a)
                    if bnd > a:
                        lo_cols.append(int(u[a]))
                        hi_cols.append(int(u[bnd - 1]))
                maxlen = max(max(lens), 1)
                if maxlen > GSLOTS:
                    ok = False
                    break
                cmax = max(hi_cols) if hi_cols else 0
                cmin = min(lo_cols) if lo_cols else 0
                if cmax < SPLIT:
                    hi = False
                elif cmin >= HIBASE:
                    hi = True
                else:
                    ok = False
                    break
                nidx = max(16, 16 * _cdiv(maxlen, 16))
                glist.append((_cdiv(maxlen, P), hi, nidx))
            if ok:
                GROUPS.append(tuple(glist))
                break
            ng += 1
            assert ng <= 8, f"bin position {p}: cannot partition columns"
    NCH = [sum(g[0] for g in GROUPS[p]) for p in range(BINS_PER_CORE)]
    F = sum(NCH)

    x_bf = np.ascontiguousarray(x, dtype=np.float32).astype(bfloat16)

    in_maps = []
    for c in range(N_CORES):
        idx_parts = []
        # S_all[t, chunk*P + r] = sum of vals of edges whose column sits in
        # gather slot (t, chunk) and whose in-bin row is r
        pos_parts, sval_parts = [], []
        off = 0
        for p in range(BINS_PER_CORE):
            g = 40 * c + p
            s = int(starts[g])
            cnt = int(n_tot[g])
            u, inv = UQ[g]
            ng = len(GROUPS[p])
            # slot index of each distinct col: group-local position plus
            # padded offset of its group
            slot_of_u = np.empty(max(len(u), 1), np.int64)
            goff = 0
            for j, (nchj, hi, nidx) in enumerate(GROUPS[p]):
                a = (len(u) * j) // ng
                bnd = (len(u) * (j + 1)) // ng
                base = HIBASE if hi else 0
                padded = np.zeros(nchj * P, np.int32)
                if bnd > a:
                    gi = u[a:bnd] - base
                    assert gi.min() >= 0 and gi.max() < SPLIT, (p, j)
                    padded[: bnd - a] = gi
                    slot_of_u[a:bnd] = goff + np.arange(bnd - a)
                # wrapped int16 idx layout: idx i at [i%16, i//16], x8
                w = padded.reshape(-1, 16).T.astype(np.int16)
                idx_parts.append(np.tile(w, (8, 1)))
                goff += nchj * P
            nch = NCH[p]
            if cnt:
                slot = slot_of_u[inv]
                t = slot % P
                ch = off + slot // P
                pos_parts.append(t * (F * P) + ch * P + ri_s[s : s + cnt])
                sval_parts.append(val_s[s : s + cnt])
            off += nch
        idx_np = np.concatenate(idx_parts, axis=1)
        s_flat = np.bincount(np.concatenate(pos_parts),
                             weights=np.concatenate(sval_parts),
                             minlength=P * F * P)
        sall_np = s_flat.reshape(P, F * P).astype(bfloat16)
        FA = sum(NCH[:IDXA_BINS])
        in_maps.append({
            "x": x_bf,
            "idxa": np.ascontiguousarray(idx_np[:, : FA * 8]),
            "idxb": np.ascontiguousarray(idx_np[:, FA * 8 :]),
            "sall": sall_np,
            "biasrow": np.asarray(bias, np.float32).reshape(1, -1).astype(
                bfloat16),
        })
    return tuple(GROUPS), in_maps


def _run(x, edge_rows, edge_cols, adj_vals, bias, trace=False, trace_cores=None):
    from concourse.bass_utils import run_bass_kernel_spmd

    GROUPS, in_maps = _preprocess(x, edge_rows, edge_cols, adj_vals, bias)
    if GROUPS not in _plan_cache:
        _plan_cache[GROUPS] = _build_program(list(GROUPS))
    nc = _plan_cache[GROUPS]
    kw = {}
    if trace:
        kw["trace"] = True
        if trace_cores is not None:
            kw["trace_cores"] = trace_cores
    res = run_bass_kernel_spmd(nc, in_maps, core_ids=list(range(N_CORES)), **kw)
    out = np.concatenate([res.results[c]["out"] for c in range(N_CORES)], axis=0)
    return out[:N_NODES].astype(np.float32), res


def kernel(x, edge_rows, edge_cols, adj_vals, bias):
    out, _ = _run(np.asarray(x), np.asarray(edge_rows), np.asarray(edge_cols),
                  np.asarray(adj_vals), np.asarray(bias))
    return out
